# revision 1
# baseline (speedup 1.0000x reference)
"""Trainium2 Bass kernel for nn_BigNet (gnn_message_passing).

The reference network pools the INPUT node features x (the original model
never reassigns x before pooling -- reproduced faithfully there), so the
output only depends on:
    x = emb_weight[global_idx] + acts @ pe_W + pe_b        [N, 256]
    pooled = segment_sum(x, batch, 64)                     [64, 256]
    z = relu(pooled @ fc1_W + fc1_b)                       [64, 512]
    out = log_softmax(z @ fc2_W + fc2_b)                   [64, 978]
The CGConv/GAT stack is dead code w.r.t. the output and is skipped.

Sharding (data parallel over the batch dimension, graph-aligned, weights
and embedding table replicated), per core i of 8 owning graphs 8i..8i+8:
  - scans the whole bf16 embedding table sequentially (pre-tiled layout,
    HWDGE at line rate) accumulating psumA[8, 256] += cnt_tile.T @ emb_tile,
    where cnt[v, s] counts occurrences of vocab row v in own graph s
    (host-built histogram; counts are small integers, exact in bf16)
  - acts pooling runs on the otherwise-idle Scalar engine: per-own-graph
    zero-padded segments of [acts0, acts1, 1] are summed in fp32 by
    activation(Copy, accum_out) into pat [3, 8], which feeds fc1 directly
    as the lhsT of the host-folded [pe_W; pe_b] @ fc1_W term
  - MLP head (bf16 weights, f32 psum) + log_softmax (no max shift; the
    logits are O(1) by construction) -> out [8, 978]
The host only builds index/layout arrays and concatenates core outputs.
"""


from contextlib import ExitStack

import ml_dtypes
import numpy as np

import concourse.bacc as bacc
import concourse.mybir as mybir
import concourse.tile as tile
from concourse.bass_utils import run_bass_kernel_spmd
from concourse.masks import make_identity

F32 = mybir.dt.float32
BF16 = mybir.dt.bfloat16
AX = mybir.AxisListType
ALU = mybir.AluOpType
ACTF = mybir.ActivationFunctionType

NCORES = 8
N_GRAPHS = 64
GPC = N_GRAPHS // NCORES


def _roundup(x, m):
    return ((x + m - 1) // m) * m


def _tile128(a, width):
    r = a.shape[0]
    return np.ascontiguousarray(a.reshape(r // 128, 128, width).transpose(1, 0, 2))


def _bf16(a):
    return a.astype(ml_dtypes.bfloat16)


def _prep_inputs(inputs):
    gi = np.asarray(inputs["global_idx"]).astype(np.int64).ravel()
    acts = np.asarray(inputs["acts"], dtype=np.float32)
    batch = np.asarray(inputs["batch"]).astype(np.int64).ravel()
    emb = np.ascontiguousarray(np.asarray(inputs["emb_weight"], dtype=np.float32))
    pe_W = np.asarray(inputs["pe_W"], dtype=np.float32)
    pe_b = np.asarray(inputs["pe_b"], dtype=np.float32).ravel()
    fc1_W = np.ascontiguousarray(np.asarray(inputs["fc1_W"], dtype=np.float32))
    fc1_b = np.asarray(inputs["fc1_b"], dtype=np.float32).ravel()
    fc2_W = np.ascontiguousarray(np.asarray(inputs["fc2_W"], dtype=np.float32))
    fc2_b = np.asarray(inputs["fc2_b"], dtype=np.float32).ravel()

    V, D = emb.shape
    H = fc1_W.shape[1]
    OUT = fc2_W.shape[1]
    VSF = _roundup(V, 128)

    bounds = np.searchsorted(batch, np.arange(0, N_GRAPHS + 1, GPC))
    seglens = np.searchsorted(batch, np.arange(1, N_GRAPHS + 1)) - np.searchsorted(
        batch, np.arange(N_GRAPHS)
    )
    SEGPAD = max(2, _roundup(int(seglens.max()), 2))

    cfg = dict(V=V, D=D, H=H, OUT=OUT, VSF=VSF, SEGPAD=SEGPAD)

    emb_pad = np.zeros((VSF, D), dtype=np.float32)
    emb_pad[:V] = emb
    pe_fc1 = np.vstack([pe_W, pe_b.reshape(1, D)]).astype(np.float32) @ fc1_W  # [3, H]

    shared = dict(
        emb=_bf16(_tile128(emb_pad, D)),
        fc1w=_bf16(_tile128(fc1_W, H)),
        fc1b=_bf16(fc1_b.reshape(1, H)),
        fc2w=_bf16(_tile128(fc2_W, OUT)),
        fc2b=_bf16(fc2_b.reshape(1, OUT)),
        pefc1=np.ascontiguousarray(pe_fc1),
    )

    in_maps = []
    for i in range(NCORES):
        sl = slice(int(bounds[i]), int(bounds[i + 1]))
        b_c = batch[sl] - i * GPC
        cnt = np.zeros((VSF, GPC), dtype=np.float32)
        np.add.at(cnt, (gi[sl], b_c), 1.0)

        a3seg = np.zeros((3, GPC * SEGPAD), dtype=np.float32)
        for s in range(GPC):
            g0 = int(bounds[i] + 0)
            lo = int(np.searchsorted(batch, i * GPC + s))
            hi = int(np.searchsorted(batch, i * GPC + s + 1))
            ln = hi - lo
            a3seg[0, s * SEGPAD : s * SEGPAD + ln] = acts[lo:hi, 0]
            a3seg[1, s * SEGPAD : s * SEGPAD + ln] = acts[lo:hi, 1]
            a3seg[2, s * SEGPAD : s * SEGPAD + ln] = 1.0

        m = dict(shared)
        m["cnt8"] = _bf16(_tile128(cnt, GPC))
        m["a3seg"] = a3seg
        in_maps.append(m)
    return in_maps, cfg


def _declare_tensors(nc, cfg):
    D, H, OUT = cfg["D"], cfg["H"], cfg["OUT"]
    VSF, SEGPAD = cfg["VSF"], cfg["SEGPAD"]
    VT = VSF // 128

    def inp(name, shape, dt=F32):
        return nc.dram_tensor(name, shape, dt, kind="ExternalInput").ap()

    ins = dict(
        emb=inp("emb", [128, VT, D], BF16),
        cnt8=inp("cnt8", [128, VT, GPC], BF16),
        a3seg=inp("a3seg", [3, GPC * SEGPAD]),
        fc1w=inp("fc1w", [128, D // 128, H], BF16),
        fc1b=inp("fc1b", [1, H], BF16),
        fc2w=inp("fc2w", [128, H // 128, OUT], BF16),
        fc2b=inp("fc2b", [1, OUT], BF16),
        pefc1=inp("pefc1", [3, H]),
    )
    out = nc.dram_tensor("out", [GPC, OUT], F32, kind="ExternalOutput").ap()
    return ins, out


def _build_kernel(tc, outs, ins, cfg):
    nc = tc.nc
    D, H, OUT = cfg["D"], cfg["H"], cfg["OUT"]
    VSF, SEGPAD = cfg["VSF"], cfg["SEGPAD"]
    G = GPC
    VT = VSF // 128
    DC, HC = D // 128, H // 128
    VCHUNK = 16  # emb tiles per DMA chunk
    osplit = []
    c0 = 0
    while c0 < OUT:
        w = min(512, OUT - c0)
        osplit.append((c0, w))
        c0 += w

    out = outs["out"]

    with ExitStack() as ctx:
        cpool = ctx.enter_context(tc.tile_pool(name="const", bufs=1))
        wpool = ctx.enter_context(tc.tile_pool(name="work", bufs=1))
        ppool = ctx.enter_context(tc.tile_pool(name="pacc", bufs=1, space="PSUM"))
        tpool = ctx.enter_context(tc.tile_pool(name="ptrans", bufs=2, space="PSUM"))
        hpool = ctx.enter_context(tc.tile_pool(name="phead", bufs=1, space="PSUM"))

        # ---- early loads; split across the two HWDGE engines ----
        cnt_t = cpool.tile([128, VT, G], BF16)
        nc.scalar.dma_start(out=cnt_t[:], in_=ins["cnt8"][:])
        a3_t = cpool.tile([3, G * SEGPAD], F32)
        nc.scalar.dma_start(out=a3_t[:], in_=ins["a3seg"][:])
        # one pool tile per emb chunk so the first matmuls start as soon as
        # chunk 0 lands (a single big tile would serialize on ALL chunk DMAs)
        nchunks = (VT + VCHUNK - 1) // VCHUNK
        es_chunks = []
        for ci in range(nchunks):
            c0_ = ci * VCHUNK
            c1_ = min(VT, c0_ + VCHUNK)
            esc = cpool.tile([128, c1_ - c0_, D], BF16, tag=f"es{ci}")
            nc.sync.dma_start(out=esc[:], in_=ins["emb"][:, c0_:c1_, :])
            es_chunks.append((c0_, c1_, esc))

        ident = cpool.tile([128, 128], F32)
        make_identity(nc, ident[:])
        ones = cpool.tile([1, G], BF16)
        nc.vector.memset(ones[:], 1.0)

        # ---- pooled emb: psumA[s, :] += sum_v cnt[v, s] * emb[v, :] ----
        psumA = ppool.tile([G, D], F32)
        for c0_, c1_, esc in es_chunks:
            for t in range(c0_, c1_):
                nc.tensor.matmul(
                    psumA[:],
                    lhsT=cnt_t[:, t, :],
                    rhs=esc[:, t - c0_, :],
                    start=(t == 0),
                    stop=(t == VT - 1),
                )

        # ---- acts pooling: per-graph segment sums on the Scalar engine ----
        pat = wpool.tile([3, G], F32)
        scr = wpool.tile([3, SEGPAD], F32)
        for s in range(G):
            nc.scalar.activation(
                scr[:],
                a3_t[:, s * SEGPAD : (s + 1) * SEGPAD],
                ACTF.Copy,
                accum_out=pat[:, s : s + 1],
            )

        # prewarm ACT tables for Exp/Ln after the Copy-accumulation ops so
        # the tail's exp/ln don't pay a table reload
        warm = wpool.tile([1, 2], F32)
        nc.vector.memset(warm[:], 1.0)
        nc.scalar.activation(warm[:], warm[:], ACTF.Exp)
        nc.scalar.activation(warm[:], warm[:], ACTF.Ln)

        # ---- late loads: head weights ----
        fc1w_t = cpool.tile([128, DC, H], BF16)
        nc.sync.dma_start(out=fc1w_t[:], in_=ins["fc1w"][:])
        fc1b_t = cpool.tile([1, H], BF16)
        nc.sync.dma_start(out=fc1b_t[:], in_=ins["fc1b"][:])
        fc2w_t = cpool.tile([128, HC, OUT], BF16)
        nc.scalar.dma_start(out=fc2w_t[:], in_=ins["fc2w"][:])
        fc2b_t = cpool.tile([1, OUT], BF16)
        nc.scalar.dma_start(out=fc2b_t[:], in_=ins["fc2b"][:])
        pefc1_t = cpool.tile([3, H], F32)
        nc.sync.dma_start(out=pefc1_t[:], in_=ins["pefc1"][:])

        # ---- fc1: z1 = relu(pooled @ fc1_W + pat.T @ pe_fc1 + fc1_b) ----
        pooled = wpool.tile([G, D], F32)
        nc.vector.tensor_copy(out=pooled[:], in_=psumA[:])
        pT = wpool.tile([128, DC, G], BF16)
        for c in range(DC):
            tp = tpool.tile([128, G], F32, tag="tp")
            nc.tensor.transpose(
                out=tp[:],
                in_=pooled[:, c * 128 : (c + 1) * 128],
                identity=ident[:G, :G],
            )
            nc.vector.tensor_copy(out=pT[:, c, :], in_=tp[:])

        z1p = hpool.tile([G, H], F32, tag="z1")
        for c in range(DC):
            nc.tensor.matmul(
                z1p[:], lhsT=pT[:, c, :], rhs=fc1w_t[:, c, :], start=(c == 0), stop=False
            )
        nc.tensor.matmul(z1p[:], lhsT=pat[:], rhs=pefc1_t[:], start=False, stop=False)
        nc.tensor.matmul(
            z1p[:], lhsT=ones[:, :G], rhs=fc1b_t[:], start=False, stop=True
        )
        z1 = wpool.tile([G, H], F32)
        nc.vector.tensor_scalar_max(z1[:], z1p[:], 0.0)

        # ---- fc2 (z2 stays in PSUM; softmax reads it there) ----
        zT = wpool.tile([128, HC, G], BF16)
        for c in range(HC):
            tp = tpool.tile([128, G], F32, tag="tp")
            nc.tensor.transpose(
                out=tp[:], in_=z1[:, c * 128 : (c + 1) * 128], identity=ident[:G, :G]
            )
            nc.vector.tensor_copy(out=zT[:, c, :], in_=tp[:])
        z2ps = []
        for c0_, w in osplit:
            z2p = hpool.tile([G, w], F32, tag=f"z2_{c0_}")
            for c in range(HC):
                nc.tensor.matmul(
                    z2p[:],
                    lhsT=zT[:, c, :],
                    rhs=fc2w_t[:, c, c0_ : c0_ + w],
                    start=(c == 0),
                    stop=False,
                )
            nc.tensor.matmul(
                z2p[:],
                lhsT=ones[:, :G],
                rhs=fc2b_t[:, c0_ : c0_ + w],
                start=False,
                stop=True,
            )
            z2ps.append((c0_, w, z2p))

        # ---- log_softmax without max shift (logits are O(1)) ----
        escr = wpool.tile([G, 512], F32)
        ssum = wpool.tile([G, len(z2ps)], F32)
        for j, (c0_, w, z2p) in enumerate(z2ps):
            nc.scalar.activation(
                escr[:, :w], z2p[:], ACTF.Exp, accum_out=ssum[:, j : j + 1]
            )
        s_all = wpool.tile([G, 1], F32)
        nc.vector.tensor_reduce(out=s_all[:], in_=ssum[:], axis=AX.X, op=ALU.add)
        ls = wpool.tile([G, 1], F32)
        nc.scalar.activation(ls[:], s_all[:], ACTF.Ln)
        o = wpool.tile([G, OUT], F32)
        for c0_, w, z2p in z2ps:
            nc.vector.tensor_scalar(
                out=o[:, c0_ : c0_ + w],
                in0=z2p[:],
                scalar1=ls[:, 0:1],
                scalar2=None,
                op0=ALU.subtract,
            )
        nc.sync.dma_start(out=out[:], in_=o[:])


def build_program(cfg):
    nc = bacc.Bacc("TRN2", debug=False, num_devices=NCORES)
    ins, out_ap = _declare_tensors(nc, cfg)
    with tile.TileContext(nc, num_cores=NCORES) as tc:
        _build_kernel(tc, {"out": out_ap}, ins, cfg)
    nc.compile()
    return nc


def run(inputs, **spmd_kwargs):
    in_maps, cfg = _prep_inputs(inputs)
    nc = build_program(cfg)
    res = run_bass_kernel_spmd(nc, in_maps, core_ids=list(range(NCORES)), **spmd_kwargs)
    full = np.concatenate([res.results[i]["out"] for i in range(NCORES)], axis=0)
    return np.asarray(full, dtype=np.float32), res


def kernel(**inputs):
    out, _ = run(inputs)
    return out

